# revision 1
# baseline (speedup 1.0000x reference)
"""DGCNN edge-conv graph-feature module on Trainium2 (Bass/Tile).

Problem: for each batch (B=8): F-space KNN (k=20) over N=4096 points (C=64),
gather neighbor features, edge-MLP (128->128->64->64 with relu), max-pool
over the 20 neighbors -> (4096, 64).

Sharding: data-parallel over batch across the 8 NeuronCores (one batch each,
SPMD single NEFF).

Per-core algorithm:
  setup:
    xT (64x4096), xT2 = 2*xT, -|x_j|^2 row, padded fp16 copy of x for
    gathering, v_i = (W1c-W1e) x_i + b1 per point (fp16), transposed weights.
  per point-tile T (128 query rows):
    dist vals: val[i,j] = 2 x_i.x_j - |x_j|^2 via one fp32 matmul per
      512-candidate block (augmented contraction row carries the -|x_j|^2).
    topk: per 512-block top-8 values + indices (DVE max8/max_index from
      PSUM), then top-20-of-64 merge on DVE; per-row index compaction via
      GPSIMD local_scatter (cumsum + scatter by position).
    gather: per-group fold-DMA rewraps the 20 indices/row into the
      16-partition-wrapped order, then SBUF-source dma_gather (transposed)
      pulls the 20 neighbor features/row as (128ch x 2560 pairs) fp16.
    MLP: L1 = relu(W1e x_j + v_i) with the v_i term applied by a matmul
      against a k-repeated identity; L2/L3 matmuls with relu+bias fused into
      the ACT-engine PSUM evacuations; max-pool over k on GPSIMD; PE
      transpose to row-major; DMA out.
"""

import os
import sys

for _p in ("/opt/trn_rl_repo", "/root/.axon_site/_ro/trn_rl_repo"):
    if os.path.isdir(_p) and _p not in sys.path:
        sys.path.insert(0, _p)

import numpy as np

import concourse.bass as bass
import concourse.mybir as mybir
from concourse import bacc
from concourse.bass_utils import run_bass_kernel_spmd
from concourse.masks import make_identity
from concourse.tile import TileContext

f32 = mybir.dt.float32
f16 = mybir.dt.float16
i16 = mybir.dt.int16
u16 = mybir.dt.uint16

B, N, C, K = 8, 4096, 64, 20
C1, C2, C3 = 128, 64, 64
NT = N // 128              # point tiles per core
NBLK = N // 512            # candidate blocks per tile (topk block size 512)
NCAND = NBLK * 8           # merge candidates per row
PAIRS = 128 * K            # pairs per point tile (2560)
GROUP = 8                  # tiles per scatter/gather/pool phase group
NEG = -1e30


def build_nc(nt=NT, stage=None):
    if stage is None:
        stage = int(os.environ.get("KM_STAGE", "9"))
    nc = bacc.Bacc(None, target_bir_lowering=False)

    pts = nc.declare_dram_parameter("points", [N, C], f32, isOutput=False)
    w1 = nc.declare_dram_parameter("W1", [C1, 2 * C], f32, isOutput=False)
    b1 = nc.declare_dram_parameter("b1", [C1], f32, isOutput=False)
    w2 = nc.declare_dram_parameter("W2", [C2, C1], f32, isOutput=False)
    b2 = nc.declare_dram_parameter("b2", [C2], f32, isOutput=False)
    w3 = nc.declare_dram_parameter("W3", [C3, C2], f32, isOutput=False)
    b3 = nc.declare_dram_parameter("b3", [C3], f32, isOutput=False)
    out = nc.declare_dram_parameter("out", [N, C3], f32, isOutput=True)

    pool_chain = []  # GPSIMD extended-ISA ops, chained to batch ucode libraries

    with TileContext(nc) as tc:
        with tc.tile_pool(name="const", bufs=1) as cp:
            ident = cp.tile([128, 128], f32)
            make_identity(nc, ident)
            ident16 = cp.tile([128, 128], f16)
            nc.vector.tensor_copy(ident16, ident)

            # ---- load points as [p, T, c] in one big DMA
            x_sb = cp.tile([128, NT, C], f32)
            src = bass.AP(
                tensor=pts.ap().tensor, offset=0,
                ap=[[C, 128], [128 * C, NT], [1, C]],
            )
            nc.sync.dma_start(out=x_sb, in_=src)

            # ---- weights + biases
            w1_sb = cp.tile([C1, 2 * C], f32)
            nc.sync.dma_start(out=w1_sb, in_=w1[:, :])
            w2_sb = cp.tile([C2, C1], f32)
            nc.sync.dma_start(out=w2_sb, in_=w2[:, :])
            w3_sb = cp.tile([C3, C2], f32)
            nc.sync.dma_start(out=w3_sb, in_=w3[:, :])
            b1_row = cp.tile([1, C1], f32)
            nc.sync.dma_start(out=b1_row, in_=b1.ap().rearrange("(a c) -> a c", a=1))
            b2_col = cp.tile([C2, 1], f32)
            nc.sync.dma_start(out=b2_col, in_=b2.ap().rearrange("(c a) -> c a", a=1))
            b3_col = cp.tile([C3, 1], f32)
            nc.sync.dma_start(out=b3_col, in_=b3.ap().rearrange("(c a) -> c a", a=1))

            # slot -> 512*block offset constant for globalizing block-local idx
            offs = cp.tile([128, NCAND], u16)
            nc.gpsimd.iota(offs, pattern=[[512, NBLK], [0, 8]], base=0,
                           channel_multiplier=0)

            # ---- xTaug [65, N]: rows 0:64 = xT, row 64 = ones (lhsT + v lhsT)
            # ---- xT2aug [65, N]: rows 0:64 = 2*xT, row 64 = -|x_j|^2 (dist rhs)
            xTaug = cp.tile([C + 1, N], f32)
            xT2aug = cp.tile([C + 1, N], f32)
            nc.vector.memset(xTaug[C:C + 1, :], 1.0)
            x_f16 = cp.tile([128, NT, 128], f16)
            nc.vector.memset(x_f16, 0.0)
            v_all = cp.tile([128, NT, C1], f16)

            with tc.tile_pool(name="setup_ps", bufs=2, space="PSUM") as sps:
                # transpose W1, W2, W3
                w1t = cp.tile([2 * C, C1], f32)    # [c_in, c_out]
                p = sps.tile([128, 128], f32, tag="tp")
                nc.tensor.transpose(p, w1_sb, ident)
                nc.vector.tensor_copy(w1t, p)
                w1t16 = cp.tile([2 * C, C1], f16)
                nc.vector.tensor_copy(w1t16, w1t)

                w2t16 = cp.tile([C1, C2], f16)
                p = sps.tile([128, 128], f32, tag="tp")
                nc.tensor.transpose(p[:, 0:C2], w2_sb, ident[0:C2, 0:C2])
                nc.vector.tensor_copy(w2t16, p[0:C1, 0:C2])

                w3t16 = cp.tile([C2, C3], f16)
                p = sps.tile([128, 128], f32, tag="tp")
                nc.tensor.transpose(p[0:C2, 0:C3], w3_sb[:, :], ident[0:C2, 0:C2])
                nc.vector.tensor_copy(w3t16, p[0:C2, 0:C3])

                # Wv_aug [65, C1]: rows 0:64 = (W1c - W1e)^T, row 64 = b1
                wv = cp.tile([C + 1, C1], f32)
                delta = cp.tile([C1, C], f32)
                nc.vector.tensor_tensor(
                    out=delta, in0=w1_sb[:, C:2 * C], in1=w1_sb[:, 0:C],
                    op=mybir.AluOpType.subtract)
                p = sps.tile([128, 128], f32, tag="tp")
                nc.tensor.transpose(p[0:C, :], delta, ident)
                nc.vector.tensor_copy(wv[0:C, :], p[0:C, :])
                nc.sync.dma_start(out=wv[C:C + 1, :],
                                  in_=b1.ap().rearrange("(a c) -> a c", a=1))

                # per point-tile: transpose x, fill xTaug/xT2aug/x_f16
                for t in range(NT):
                    p = sps.tile([128, 128], f32, tag="tp")
                    nc.tensor.transpose(p[0:C, :], x_sb[:, t, :], ident)
                    sl = slice(t * 128, (t + 1) * 128)
                    nc.vector.tensor_copy(xTaug[0:C, sl], p[0:C, :])
                    nc.vector.tensor_scalar_mul(xT2aug[0:C, sl], p[0:C, :], 2.0)
                    nc.vector.tensor_copy(x_f16[:, t, 0:C], x_sb[:, t, :])

                # -|x_j|^2 row: square then columns summed by ones-matmul
                xsq = cp.tile([C, N], f32)
                nc.vector.tensor_mul(xsq, xTaug[0:C, :], xTaug[0:C, :])
                negones = cp.tile([C, 1], f32)
                nc.vector.memset(negones, -1.0)
                sqrow = cp.tile([1, N], f32)
                for b in range(NBLK):
                    p = sps.tile([1, 512], f32, tag="sq")
                    nc.tensor.matmul(p, negones, xsq[:, b * 512:(b + 1) * 512],
                                     start=True, stop=True)
                    nc.vector.tensor_copy(sqrow[0:1, b * 512:(b + 1) * 512], p)
                nc.sync.dma_start(out=xT2aug[C:C + 1, :], in_=sqrow)

                # v tiles: (x W_v + b1) per point, fp16, [i, T, ch]
                for t in range(NT):
                    p = sps.tile([128, 128], f32, tag="tp")
                    nc.tensor.matmul(
                        p, xTaug[:, t * 128:(t + 1) * 128], wv,
                        start=True, stop=True)
                    nc.vector.tensor_copy(v_all[:, t, :], p)

            # ---- per-tile index list + wrapped gather lists
            ilist = cp.tile([128, nt, 32], i16)
            wrap = cp.tile([128, nt, 160], i16)

            with tc.tile_pool(name="topk", bufs=2) as tk, \
                 tc.tile_pool(name="mlp", bufs=3) as mp, \
                 tc.tile_pool(name="h2p", bufs=2) as h2p, \
                 tc.tile_pool(name="h3p", bufs=3) as h3p, \
                 tc.tile_pool(name="outp", bufs=2) as op_, \
                 tc.tile_pool(name="ps_dist", bufs=2, space="PSUM") as psd, \
                 tc.tile_pool(name="ps_l1", bufs=1, space="PSUM") as ps1, \
                 tc.tile_pool(name="ps_l2", bufs=1, space="PSUM") as ps2, \
                 tc.tile_pool(name="ps_l3", bufs=1, space="PSUM") as ps3, \
                 tc.tile_pool(name="ps_ot", bufs=1, space="PSUM") as pso:

                def topk_tile(t):
                    m_sb = tk.tile([128, NCAND], f32, tag="m_sb")
                    lidx = tk.tile([128, NCAND], u16, tag="lidx")
                    for b in range(NBLK):
                        pd = psd.tile([128, 512], f32, tag="dist")
                        nc.tensor.matmul(
                            pd, xTaug[:, t * 128:(t + 1) * 128],
                            xT2aug[:, b * 512:(b + 1) * 512],
                            start=True, stop=True)
                        nc.vector.max(out=m_sb[:, b * 8:(b + 1) * 8], in_=pd)
                        nc.vector.max_index(
                            out=lidx[:, b * 8:(b + 1) * 8],
                            in_max=m_sb[:, b * 8:(b + 1) * 8], in_values=pd)
                    if stage < 3:
                        return
                    # merge: top-20 of 64 -> per-row compacted gather list
                    work = tk.tile([128, NCAND], f32, tag="work")
                    nc.vector.tensor_copy(work, m_sb)
                    t8 = tk.tile([128, 8], f32, tag="t8")
                    for _ in range(2):
                        nc.vector.max(out=t8, in_=work)
                        nc.vector.match_replace(
                            out=work, in_to_replace=t8, in_values=work,
                            imm_value=NEG)
                    nc.vector.max(out=t8, in_=work)
                    qual = tk.tile([128, NCAND], f32, tag="qual")
                    # qual = m_sb >= v20 (v20 = 4th value of 3rd octet)
                    nc.vector.tensor_scalar(
                        qual, m_sb, t8[:, 3:4], scalar2=None,
                        op0=mybir.AluOpType.is_ge)
                    pos = tk.tile([128, NCAND], f32, tag="pos")
                    nc.vector.tensor_tensor_scan(
                        out=pos, data0=qual, data1=qual, initial=0.0,
                        op0=mybir.AluOpType.add, op1=mybir.AluOpType.bypass)
                    # position = qual*cumsum - 1 (invalid -> -1), clamp <= 31
                    nc.vector.tensor_mul(pos, pos, qual)
                    nc.vector.tensor_scalar(
                        pos, pos, 1.0, scalar2=31.0,
                        op0=mybir.AluOpType.subtract, op1=mybir.AluOpType.min)
                    pidx = tk.tile([128, NCAND], i16, tag="pidx")
                    nc.vector.tensor_copy(pidx, pos)
                    gidx = tk.tile([128, NCAND], i16, tag="gidx")
                    nc.vector.tensor_tensor(
                        out=gidx, in0=lidx, in1=offs,
                        op=mybir.AluOpType.add)
                    sc = nc.gpsimd.local_scatter(
                        out_ap=ilist[:, t, :], data_ap=gidx, idxs_ap=pidx,
                        channels=128, num_elems=32, num_idxs=NCAND)
                    pool_chain.append(sc.ins)

                def fold_group(g, tiles):
                    # rewrap idx lists: wrap[q, T, h*20+k] = ilist[16h+q, T, k]
                    for h in range(8):
                        nc.sync.dma_start(
                            out=wrap[0:16, tiles[0]:tiles[-1] + 1,
                                     h * 20:(h + 1) * 20],
                            in_=ilist[16 * h:16 * (h + 1),
                                      tiles[0]:tiles[-1] + 1, 0:20])
                    for g2 in range(1, 8):
                        nc.sync.dma_start(
                            out=wrap[16 * g2:16 * (g2 + 1),
                                     tiles[0]:tiles[-1] + 1, :],
                            in_=wrap[0:16, tiles[0]:tiles[-1] + 1, :])

                def gather_tile(t):
                    xg = mp.tile([128, 1, PAIRS], f16, tag="xg")
                    gi = nc.gpsimd.dma_gather(
                        out_ap=xg,
                        in_ap=x_f16.rearrange("p t c -> p (t c)"),
                        idxs_ap=wrap[:, t, :],
                        num_idxs=PAIRS,
                        num_idxs_reg=PAIRS,
                        elem_size=128,
                        transpose=True,
                        sbuf_tokens_per_rank=128,
                        sbuf_free_dim_per_rank=256,
                        sbuf_free_dim_pad_per_rank=0,
                        sbuf_byte_offset=0,
                        single_packet=False,
                    )
                    pool_chain.append(gi.ins)
                    return xg

                def mlp_tile(t, xg):
                    h2 = h2p.tile([C2, PAIRS], f16, tag="h2")
                    for c in range(8):  # 320-pair chunks (one h each)
                        sl = slice(c * 320, (c + 1) * 320)
                        p1 = ps1.tile([C1, 320], f32, tag="l1")
                        rep = bass.AP(
                            tensor=ident16.tensor,
                            offset=ident16.offset + 16 * c * ident16.ap[-1][0],
                            ap=[ident16.ap[0], [0, K], [ident16.ap[-1][0], 16]],
                        )
                        nc.tensor.matmul(p1, v_all[:, t, :], rep,
                                         start=True, stop=False)
                        nc.tensor.matmul(p1, w1t16, xg[:, 0, sl],
                                         start=False, stop=True)
                        h1 = mp.tile([C1, 320], f16, tag="h1")
                        nc.scalar.activation(
                            h1, p1, mybir.ActivationFunctionType.Relu)
                        p2 = ps2.tile([C2, 320], f32, tag="l2")
                        nc.tensor.matmul(p2, w2t16, h1, start=True, stop=True)
                        nc.scalar.activation(
                            h2[:, sl], p2, mybir.ActivationFunctionType.Relu,
                            bias=b2_col)
                    h3 = h3p.tile([C3, PAIRS], f16, tag="h3")
                    for half in range(2):
                        p3 = ps3.tile([C3, 1280], f32, tag="l3")
                        for q0, q1 in ((0, 512), (512, 1024), (1024, 1280)):
                            nc.tensor.matmul(
                                p3[:, q0:q1], w3t16,
                                h2[:, half * 1280 + q0:half * 1280 + q1],
                                start=True, stop=True)
                        nc.scalar.activation(
                            h3[:, half * 1280:(half + 1) * 1280], p3,
                            mybir.ActivationFunctionType.Relu, bias=b3_col)
                    return h3

                def pool_out_tile(t, h3):
                    pooled = op_.tile([C3, 128], f32, tag="pooled")

                    def kview(src, nk, k0, kn, dt_tile=None):
                        # view [64, 8h, kn(of nk), 16q] of a (h,k,q) tensor
                        s = src.ap[-1][0]
                        return bass.AP(
                            tensor=src.tensor,
                            offset=src.offset + k0 * 16 * s,
                            ap=[src.ap[0], [s * 16 * nk, 8], [s * 16, kn],
                                [s, 16]],
                        )

                    m10 = h3p.tile([C3, 1280], f16, tag="m10")
                    nc.vector.tensor_max(
                        m10, kview(h3, K, 0, 10), kview(h3, K, 10, 10))
                    m5 = h3p.tile([C3, 640], f16, tag="m5")
                    nc.vector.tensor_max(
                        m5, kview(m10, 10, 0, 5), kview(m10, 10, 5, 5))
                    m2 = h3p.tile([C3, 256], f16, tag="m2")
                    nc.vector.tensor_max(
                        m2, kview(m5, 5, 0, 2), kview(m5, 5, 2, 2))
                    m1 = h3p.tile([C3, 128], f16, tag="m1")
                    nc.vector.tensor_max(
                        m1, kview(m2, 2, 0, 1), kview(m2, 2, 1, 1))
                    nc.vector.tensor_max(pooled, m1, kview(m5, 5, 4, 1))
                    po = pso.tile([128, C3], f32, tag="ot")
                    nc.tensor.transpose(po, pooled, ident[0:C3, 0:C3])
                    osb = op_.tile([128, C3], f32, tag="osb")
                    nc.vector.tensor_copy(osb, po)
                    nc.sync.dma_start(
                        out=out[t * 128:(t + 1) * 128, :], in_=osb)

                n_groups = (nt + GROUP - 1) // GROUP
                for g in range(n_groups):
                    tiles = list(range(g * GROUP, min((g + 1) * GROUP, nt)))
                    for t in tiles:
                        topk_tile(t)
                    if stage >= 4:
                        fold_group(g, tiles)
                    if stage >= 5:
                        xgs = {t: gather_tile(t) for t in tiles}
                    if stage >= 6:
                        for t in tiles:
                            h3 = mlp_tile(t, xgs[t])
                            pool_out_tile(t, h3)

        # order GPSIMD extended-ISA ops to batch ucode-library reloads
        if os.environ.get("KM_CHAIN", "1") == "1":
            from concourse.tile_rust import add_dep_helper
            for a, b_ in zip(pool_chain, pool_chain[1:]):
                add_dep_helper(b_, a, sync=False, reason="gpsimd library batching")

    return nc


_nc_cache = None
LAST_EXEC_NS = None


def kernel(points, W1, b1, W2, b2, W3, b3):
    global _nc_cache
    if _nc_cache is None:
        _nc_cache = build_nc()
        # Bacc defers register allocation to compile(); the PJRT path
        # serializes the module as-is, so finalize before running.
        _nc_cache.finalize()
    nc = _nc_cache
    common = {
        "W1": np.ascontiguousarray(W1, dtype=np.float32),
        "b1": np.ascontiguousarray(b1, dtype=np.float32),
        "W2": np.ascontiguousarray(W2, dtype=np.float32),
        "b2": np.ascontiguousarray(b2, dtype=np.float32),
        "W3": np.ascontiguousarray(W3, dtype=np.float32),
        "b3": np.ascontiguousarray(b3, dtype=np.float32),
    }
    in_maps = [
        dict(common, points=np.ascontiguousarray(points[b], dtype=np.float32))
        for b in range(B)
    ]
    trace = os.environ.get("BASS_TRACE", "0") == "1"
    res = run_bass_kernel_spmd(nc, in_maps, list(range(B)), trace=trace)
    global LAST_EXEC_NS
    LAST_EXEC_NS = res.exec_time_ns
    return np.stack([res.results[b]["out"] for b in range(B)], axis=0)


if __name__ == "__main__":
    pts = np.random.randn(B, N, C).astype(np.float32)
    W1_ = (np.random.randn(C1, 2 * C) * 0.05).astype(np.float32)
    W2_ = (np.random.randn(C2, C1) * 0.05).astype(np.float32)
    W3_ = (np.random.randn(C3, C2) * 0.05).astype(np.float32)
    z1, z2, z3 = (np.zeros(C1, np.float32), np.zeros(C2, np.float32),
                  np.zeros(C3, np.float32))
    o = kernel(pts, W1_, z1, W2_, z2, W3_, z3)
    print(o.shape, o.dtype)

